# revision 2
# baseline (speedup 1.0000x reference)
"""LogicLayer Trainium2 kernel, v2.

out[b, n] = sum_k softmax(w[n])_k * gate_k(a1, a2),  a1 = x[b, i1[n]], a2 = x[b, i2[n]]

All 16 gates are affine in {1, a1, a2, a1*a2}:
    out = A0 + A1*a1 + A2*a2 + Ap*a1*a2,   A = softmax(w) @ C   (host-computed)

Factored per neuron as
    u = s_u1*g2 + s_u2    (DVE tensor_scalar, 4x fp16 mode)
    v = s_v1*g2 + s_v2    (ACT Identity with AP scale/bias)
    out = u*g1 + v        (2 DVE tensor_tensor ops, 2x fp16 mode)

Device plan (8 cores, neuron-sharded: 1024 neurons x full 2048 batch each):
  MODE "p16": x.T stored as fp16 bytes VIEWED as f32 [8192, 1024] in HBM; the
    f32 indirect gather moves fp16 bytes (proven f32 descriptor path), compute
    reads the SBUF tile through a .bitcast(f16) view.
  MODE "u8c": x.T stored as uint8 (round(255 x)); indirect gather casts
    u8 -> fp16 during DMA; 1/255 factors folded into the coefficients.
  Tapered slot groups [2,2,2,1,1]; group tiles are FLAT [128, 2*csl*row] so
  every gather dest is a clean 2-level AP (3-level APs mis-address in the
  SWDGE ucode). Neurons placed sorted by i2 per core for HBM locality.
  Output written neuron-major [1024, 2048] fp16; host reassembles.
"""

import numpy as np

BATCH = 2048
NIN = 8192
NNEUR = 8192
NCORES = 8
NN = NNEUR // NCORES   # 1024 neurons per core
NB = BATCH
SLOTS = NN // 128      # 8
GROUPS = [2, 2, 2, 1, 1]  # slots per pipeline group (tapered tail)

MODE = "u8c"           # "p16" | "u8c"
GCOLS = 1              # index columns per indirect_dma_start (HW supports 1 only)

# gate -> (c0, c1, c2, cp): gate_k(a1,a2) = c0 + c1*a1 + c2*a2 + cp*a1*a2
GATE_COEF = np.array(
    [
        [0, 0, 0, 0], [0, 0, 0, 1], [0, 1, 0, -1], [0, 1, 0, 0],
        [0, 0, 1, -1], [0, 0, 1, 0], [0, 1, 1, -2], [0, 1, 1, -1],
        [1, -1, -1, 1], [1, -1, -1, 2], [1, 0, -1, 0], [1, 0, -1, 1],
        [1, -1, 0, 0], [1, -1, 0, 1], [1, 0, 0, -1], [1, 0, 0, 0],
    ],
    dtype=np.float64,
)  # [16, 4] -> columns (A0, A1, A2, Ap)

_CACHE = {}


def _build_nc():
    import concourse.bacc as bacc
    import concourse.bass as bass
    import concourse.mybir as mybir
    from concourse.tile import TileContext

    f16 = mybir.dt.float16
    f32 = mybir.dt.float32
    i32 = mybir.dt.int32
    u8 = mybir.dt.uint8
    mult = mybir.AluOpType.mult
    add = mybir.AluOpType.add
    Ident = mybir.ActivationFunctionType.Identity

    if MODE == "p16":
        src_dt, src_w, tile_dt, tile_w = f32, NB // 2, f32, NB // 2
    else:  # u8c
        src_dt, src_w, tile_dt, tile_w = u8, NB, f16, NB

    nc = bacc.Bacc("TRN2")
    xt = nc.dram_tensor("xt", [NIN, src_w], src_dt, kind="ExternalInput")
    # idx col layout: group at slot s0 with csl slots ->
    #   cols [2*s0 + k] = i1 of slot s0+k, cols [2*s0 + csl + k] = i2
    idx = nc.dram_tensor("idx", [128, 2 * SLOTS], i32, kind="ExternalInput")
    # coef[p, j, s]: j = 0:s_u1 1:s_u2 2:s_v1 3:s_v2 for slot s
    coef = nc.dram_tensor("coef", [128, 4, SLOTS], f32, kind="ExternalInput")
    yt = nc.dram_tensor("yt", [NN, NB], f16, kind="ExternalOutput")

    with TileContext(nc) as tc:
        with (
            tc.tile_pool(name="fixed", bufs=1) as fixed_pool,
            tc.tile_pool(name="gath", bufs=4) as gath_pool,
            tc.tile_pool(name="uwork", bufs=4) as u_pool,
            tc.tile_pool(name="outp", bufs=4) as out_pool,
        ):
            it = fixed_pool.tile([128, 2 * SLOTS], i32)
            nc.gpsimd.dma_start(it[:], idx[:])
            ct = fixed_pool.tile([128, 4, SLOTS], f32)
            nc.sync.dma_start(ct[:], coef[:])

            s0 = 0
            for csl in GROUPS:
                # group of csl slots [s0, s0+csl): gather g2 cols first, then g1
                g = gath_pool.tile([128, 2 * csl * tile_w], tile_dt, tag="g")
                for k in range(csl):
                    nc.gpsimd.indirect_dma_start(
                        out=g[:, (csl + k) * tile_w:(csl + k + 1) * tile_w],
                        out_offset=None, in_=xt[:],
                        in_offset=bass.IndirectOffsetOnAxis(
                            ap=it[:, 2 * s0 + csl + k:2 * s0 + csl + k + 1],
                            axis=0),
                    )
                for k in range(csl):
                    nc.gpsimd.indirect_dma_start(
                        out=g[:, k * tile_w:(k + 1) * tile_w],
                        out_offset=None, in_=xt[:],
                        in_offset=bass.IndirectOffsetOnAxis(
                            ap=it[:, 2 * s0 + k:2 * s0 + k + 1], axis=0),
                    )
                gv = g[:].bitcast(f16) if MODE == "p16" else g[:]

                u = u_pool.tile([128, csl * NB], f16, tag="u")
                o = out_pool.tile([128, csl * NB], f16, tag="o")
                for k in range(csl):
                    s = s0 + k
                    g2 = gv[:, (csl + k) * NB:(csl + k + 1) * NB]
                    # u_k = s_u1*g2 + s_u2   (DVE TS, 4x)
                    nc.vector.tensor_scalar(
                        u[:, k * NB:(k + 1) * NB], g2,
                        ct[:, 0, s:s + 1], ct[:, 1, s:s + 1], mult, add,
                    )
                    # v_k = s_v1*g2 + s_v2   (ACT Identity)
                    nc.scalar.activation(
                        o[:, k * NB:(k + 1) * NB], g2, Ident,
                        bias=ct[:, 3, s:s + 1], scale=ct[:, 2, s:s + 1],
                    )
                # t = u * g1 (all csl slots, one TT, 2x)
                nc.vector.tensor_mul(u[:], u[:], gv[:, 0:csl * NB])
                # out = t + v (one TT, 2x)
                nc.vector.tensor_add(o[:], o[:], u[:])
                if csl == 1:
                    nc.sync.dma_start(yt[s0 * 128:(s0 + 1) * 128, :], o[:])
                else:
                    dst = yt[s0 * 128:(s0 + csl) * 128, :].rearrange(
                        "(s p) b -> p s b", p=128
                    )
                    nc.sync.dma_start(
                        dst, o[:].rearrange("p (s b) -> p s b", b=NB))
                s0 += csl

    nc.compile()
    return nc


def _prep_core_inputs(x, w, conn_indices):
    """Host-side shard/layout prep. Returns list of per-core input dicts."""
    if MODE == "p16":
        xt = np.ascontiguousarray(x.T).astype(np.float16).view(np.float32)
        s1, s2 = 1.0, 1.0
    else:  # u8c
        xt = np.round(np.ascontiguousarray(x.T) * 255.0).astype(np.uint8)
        s1, s2 = 1.0 / 255.0, 1.0 / (255.0 * 255.0)

    # A = softmax(w) @ C  (f64 host math)
    z = np.exp(w.astype(np.float64) - w.astype(np.float64).max(axis=1, keepdims=True))
    probs = z / z.sum(axis=1, keepdims=True)
    A = probs @ GATE_COEF  # [NNEUR, 4] = (A0, A1, A2, Ap)

    maps = []
    perms = []
    for cidx in range(NCORES):
        n0 = cidx * NN
        # place neurons sorted by i2 so each slot's gather covers a narrow
        # sorted HBM band (better DRAM locality for the random row reads)
        order = np.argsort(conn_indices[n0:n0 + NN, 1], kind="stable")
        perm = n0 + order              # device row j holds neuron perm[j]
        perms.append(perm)
        ci = conn_indices[perm]        # [NN, 2] in placement order
        Ap_ = A[perm]                  # [NN, 4]
        idx_cols = np.empty((128, 2 * SLOTS), dtype=np.int32)
        coef = np.empty((128, 4, SLOTS), dtype=np.float32)
        s0 = 0
        for csl in GROUPS:
            for k in range(csl):
                jb = (s0 + k) * 128
                idx_cols[:, 2 * s0 + k] = ci[jb:jb + 128, 0]
                idx_cols[:, 2 * s0 + csl + k] = ci[jb:jb + 128, 1]
            s0 += csl
        for s in range(SLOTS):
            As = Ap_[s * 128:(s + 1) * 128]  # [128, 4] = (A0, A1, A2, Ap)
            coef[:, 0, s] = As[:, 3] * s2   # s_u1 = Ap (scaled)
            coef[:, 1, s] = As[:, 1] * s1   # s_u2 = A1
            coef[:, 2, s] = As[:, 2] * s1   # s_v1 = A2
            coef[:, 3, s] = As[:, 0]        # s_v2 = A0
        maps.append({"xt": xt, "idx": idx_cols, "coef": coef})
    return maps, perms


def run_cores(in_maps, trace=False):
    from concourse.bass_utils import run_bass_kernel_spmd

    if "nc" not in _CACHE:
        _CACHE["nc"] = _build_nc()
    return run_bass_kernel_spmd(
        _CACHE["nc"], in_maps, core_ids=list(range(NCORES)), trace=trace
    )


def _assemble(results, perms):
    out = np.empty((BATCH, NNEUR), dtype=np.float32)
    for c in range(NCORES):
        out[:, perms[c]] = results[c]["yt"].T.astype(np.float32)
    return out


def kernel(x, w, conn_indices):
    x = np.asarray(x, dtype=np.float32)
    w = np.asarray(w, dtype=np.float32)
    conn_indices = np.asarray(conn_indices)
    in_maps, perms = _prep_core_inputs(x, w, conn_indices)
    res = run_cores(in_maps)
    return _assemble([r for r in res.results], perms)
